# revision 1
# baseline (speedup 1.0000x reference)
"""Centroid triplet loss on 8 TRN2 NeuronCores (Bass/Tile).

Data-parallel over N=8192 rows: each core gets 1024 rows (plus a
pre-transposed copy for the distance GEMM), computes local per-class
segment sums/counts on the PE (one-hot matmul), all-reduces them,
forms -2*centroid^T and ||c||^2, then computes its local 1024 x 256
distance block and the masked triplet terms. Each core emits one
partial loss sum; the host divides by N.

Self-contained: hardcodes shapes from the problem spec.
"""

import numpy as np

import concourse.bass as bass
import concourse.bacc as bacc
import concourse.mybir as mybir
from concourse import tile
from concourse.bass_utils import run_bass_kernel_spmd

N = 8192
D = 2048
C = 256
W = 8              # cores
NL = N // W        # 1024 rows per core
NB = NL // 128     # 8 row blocks per core
KD = D // 128      # 16 contraction chunks
MARGIN = 0.3
BIG = 1.0e30

F32 = mybir.dt.float32
F32R = mybir.dt.float32r
F16 = mybir.dt.float16
I32 = mybir.dt.int32
AX = mybir.AxisListType
ALU = mybir.AluOpType
ACTF = mybir.ActivationFunctionType


def _mm_view(ap, use_f32r):
    return ap.bitcast(F32R) if use_f32r else ap


def _f32v(ap, is_f32r):
    """View an f32r-declared tile as plain f32 (same 32-bit payload)."""
    return ap.bitcast(F32) if is_f32r else ap


def _finish(nc, pp, ps1, col_ap, ones_col, out_t):
    """Write a scalar derived from col_ap [128,1] to out_t (debug truncation)."""
    ps_loss = ps1.tile([1, 1], F32, name="ps_loss")
    nc.tensor.matmul(ps_loss[:], lhsT=col_ap, rhs=ones_col[:], start=True, stop=True)
    loss_sb = pp.tile([1, 1], F32, name="loss_sb")
    nc.vector.tensor_copy(loss_sb[:], ps_loss[:])
    nc.sync.dma_start(out_t[:, :], loss_sb[:])


def emit(nc, tc, emb_in, embT_in, lab_in, out_t, mm="f32", no_collective=False,
         stage=4, use_ttr=False, ar16=False):
    use_f32r = mm == "f32r"
    DT = {"f32": F32, "f32r": F32R, "f16": F16}[mm]
    mixed = mm != "f32"
    ARDT = F16 if ar16 else F32
    with (
        tc.tile_pool(name="dram", bufs=1, space="DRAM") as dpool,
        tc.tile_pool(name="persist", bufs=1) as pp,
        tc.tile_pool(name="blocks", bufs=NB) as bp,
        tc.tile_pool(name="chunks", bufs=KD) as cp,
        tc.tile_pool(name="scratch", bufs=2) as sp,
        tc.tile_pool(name="ps2", bufs=2, space="PSUM") as psp,
        tc.tile_pool(name="ps1", bufs=1, space="PSUM") as ps1,
    ):
        cc_in = dpool.tile([D + 1, C], ARDT, name="cc_in")
        cc_out = dpool.tile([D + 1, C], ARDT, name="cc_out", addr_space="Shared")

        iota_i = pp.tile([128, C], I32, name="iota_i")
        nc.gpsimd.iota(iota_i[:], pattern=[[1, C]], base=0, channel_multiplier=0)
        iota_t = pp.tile([128, C], F32, name="iota_t")
        nc.vector.tensor_copy(iota_t[:], iota_i[:])
        ones_col = pp.tile([128, 1], F32, name="ones_col")
        nc.vector.memset(ones_col[:], 1.0)
        ones_col_r = ones_col_h = None
        if mm == "f32r":
            ones_col_r = pp.tile([128, 1], F32R, name="ones_col_r")
            nc.vector.tensor_copy(ones_col_r[:], ones_col[:])
        elif mm == "f16":
            ones_col_h = pp.tile([128, 1], F16, name="ones_col_h")
            nc.vector.tensor_copy(ones_col_h[:], ones_col[:])
        ones_row = pp.tile([1, 128], F32, name="ones_row")
        nc.vector.memset(ones_row[:], 1.0)

        # ---- load local embedding rows / labels ----
        # emb tiles + the e2 scratch live in their own pool, released after
        # the segment-sum phase so the distance phase can reuse the SBUF.
        lab_t = []
        oh_t = []
        e2_t = []
        with tc.tile_pool(name="embp", bufs=NB) as ep:
            emb_t = []
            for b in range(NB):
                et = ep.tile([128, D], DT, name=f"emb{b}", tag="emb")
                src_ap = emb_in[b * 128:(b + 1) * 128, :]
                if use_f32r:
                    src_ap = src_ap.bitcast(F32R)
                nc.sync.dma_start(et[:], src_ap)
                emb_t.append(et)
                lt = bp.tile([128, 1], F32, name=f"lab{b}", tag="lab")
                nc.sync.dma_start(lt[:], lab_in[b * 128:(b + 1) * 128, :])
                lab_t.append(lt)

            # one-hot rows (f32) for the segment-sum GEMM / ap pick
            for b in range(NB):
                oh = bp.tile([128, C], DT, name=f"oh{b}", tag="oh")
                nc.vector.tensor_scalar(
                    oh[:], iota_t[:], lab_t[b][:], None, ALU.is_equal
                )
                oh_t.append(oh)

            # ||e||^2 per row (scalar engine, accumulate along free dim)
            e2scr = ep.tile([128, D], F32, name="e2scr", tag="e2scr", bufs=1)
            for b in range(NB):
                e2 = bp.tile([128, 1], F32, name=f"e2_{b}", tag="e2")
                nc.scalar.activation(
                    e2scr[:], _f32v(emb_t[b][:], use_f32r), ACTF.Square,
                    accum_out=e2[:],
                )
                e2_t.append(e2)

            if stage <= 1:
                _finish(nc, pp, ps1, e2_t[0][:], ones_col, out_t)
                return

            # ---- transposed embeddings for the distance GEMM ----
            embT_t = []
            for k in range(KD):
                tt = cp.tile([128, NL], DT, name=f"embT{k}", tag="embT")
                tsrc = embT_in[k * 128:(k + 1) * 128, :]
                if use_f32r:
                    tsrc = tsrc.bitcast(F32R)
                nc.sync.dma_start(tt[:], tsrc)
                embT_t.append(tt)

            # ---- local segment sums (transposed): sumsT[d, c] ----
            for k in range(KD):
                ps = psp.tile([128, C], F32, name=f"ps_sums{k}", tag="ps_sums")
                for b in range(NB):
                    nc.tensor.matmul(
                        ps[:],
                        lhsT=emb_t[b][:, k * 128:(k + 1) * 128],
                        rhs=oh_t[b][:],
                        start=(b == 0),
                        stop=(b == NB - 1),
                    )
                ssb = sp.tile([128, C], ARDT, name="ssb", tag="ssb", bufs=2)
                nc.vector.tensor_copy(ssb[:], ps[:])
                nc.sync.dma_start(cc_in[k * 128:(k + 1) * 128, :], ssb[:])


            # ---- local counts (as a row vector) ----
            ps_cnt = ps1.tile([1, C], F32, name="ps_cnt")
            for b in range(NB):
                nc.tensor.matmul(
                    ps_cnt[:],
                    lhsT={"f32": ones_col, "f32r": ones_col_r,
                          "f16": ones_col_h}[mm][:],
                    rhs=oh_t[b][:],
                    start=(b == 0),
                    stop=(b == NB - 1),
                )
            cnt_sb = pp.tile([1, C], ARDT, name="cnt_sb")
            nc.vector.tensor_copy(cnt_sb[:], ps_cnt[:])
            nc.sync.dma_start(cc_in[D:D + 1, :], cnt_sb[:])

        # ---- global reduction across the 8 cores ----
        if no_collective:
            nc.sync.dma_start(cc_out[:, :], cc_in[:, :])
        else:
            nc.gpsimd.collective_compute(
                "AllReduce",
                ALU.add,
                replica_groups=[list(range(W))],
                ins=[cc_in[:, :]],
                outs=[cc_out[:, :]],
            )

        # post-allreduce tensors: allocated after the emb pool is released
        with tc.tile_pool(name="postp", bufs=1) as gp:
            gs_t = gp.tile([128, KD, C], ARDT, name="gs_t")
            nc.sync.dma_start(
                gs_t[:], cc_out[0:D, :].rearrange("(k p) c -> p k c", p=128)
            )
            cnt_row = pp.tile([1, C], ARDT, name="cnt_row")
            nc.sync.dma_start(cnt_row[:], cc_out[D:D + 1, :])
            if ar16:
                cnt_row32 = pp.tile([1, C], F32, name="cnt_row32")
                nc.vector.tensor_copy(cnt_row32[:], cnt_row[:])
                cnt_row = cnt_row32

            if stage <= 2:
                _finish(nc, pp, ps1, gs_t[:, 0, 0:1], ones_col, out_t)
                return

            # -2 / counts, broadcast to all partitions
            invc_row = pp.tile([1, C], F32, name="invc_row")
            nc.vector.reciprocal(invc_row[:], cnt_row[:])
            m2invc_row = pp.tile([1, C], F32, name="m2invc_row")
            nc.vector.tensor_scalar_mul(m2invc_row[:], invc_row[:], -2.0)
            # broadcast the [1, C] row to all partitions via a K=1 outer product
            ps_bc = psp.tile([128, C], F32, name="ps_bc", tag="ps_d2")
            nc.tensor.matmul(
                ps_bc[:], lhsT=ones_row[:], rhs=m2invc_row[:], start=True, stop=True
            )
            m2invc_b = gp.tile([128, C], F32, name="m2invc_b")
            nc.vector.tensor_copy(m2invc_b[:], ps_bc[:])

            # cen_t = -2 * centroid^T, chunked [d, c]
            cen_t = gp.tile([128, KD, C], DT, name="cen_t")
            for k in range(KD):
                nc.vector.tensor_mul(cen_t[:, k, :], gs_t[:, k, :], m2invc_b[:])

            # ||c||^2 row: sum_d cen_t^2 = 4*||c||^2 -> scale by 0.25
            ps_c2 = ps1.tile([1, C], F32, name="ps_c2")
            for k in range(KD):
                sq = sp.tile([128, C], F32, name="sq", tag="sq")
                nc.scalar.activation(
                    sq[:], _f32v(cen_t[:, k, :], use_f32r), ACTF.Square
                )
                nc.tensor.matmul(
                    ps_c2[:],
                    lhsT=ones_col[:],
                    rhs=sq[:],
                    start=(k == 0),
                    stop=(k == KD - 1),
                )
            c2_row = pp.tile([1, C], F32, name="c2_row")
            nc.scalar.mul(c2_row[:], ps_c2[:], 0.25)
            ps_bc2 = psp.tile([128, C], F32, name="ps_bc2", tag="ps_d2")
            nc.tensor.matmul(
                ps_bc2[:], lhsT=ones_row[:], rhs=c2_row[:], start=True, stop=True
            )
            c2_b = gp.tile([128, C], F32, name="c2_b")
            nc.vector.tensor_copy(c2_b[:], ps_bc2[:])

            if stage <= 3:
                _finish(nc, pp, ps1, _f32v(cen_t[:, 0, 0:1], use_f32r),
                        ones_col, out_t)
                return

            # ---- distance block + triplet terms per row block ----
            terms = gp.tile([128, NB], F32, name="terms")
            nb_run = 1 if stage in (31, 32) else NB
            for b in range(nb_run):
                psd = psp.tile([128, C], F32, name=f"ps_d2_{b}", tag="ps_d2")
                for k in range(KD):
                    nc.tensor.matmul(
                        psd[:],
                        lhsT=embT_t[k][:, b * 128:(b + 1) * 128],
                        rhs=cen_t[:, k, :],
                        start=(k == 0),
                        stop=(k == KD - 1),
                    )
                # d2 = (-2 e.c) + ||c||^2 + ||e||^2, clamped at 0
                dsum = sp.tile([128, C], F32, name="dsum", tag="sq")
                nc.vector.tensor_add(dsum[:], psd[:], c2_b[:])
                d2s = sp.tile([128, C], F32, name="d2s", tag="d2s")
                nc.vector.tensor_scalar(
                    d2s[:], dsum[:], e2_t[b][:], 0.0, ALU.add, ALU.max
                )
                if stage == 31:
                    _finish(nc, pp, ps1, d2s[:, 0:1], ones_col, out_t)
                    return
                dists = sp.tile([128, C], F32, name="dists", tag="dists")
                nc.scalar.activation(dists[:], d2s[:], ACTF.Sqrt)

                msk = sp.tile([128, C], F32, name="msk", tag="msk")
                nc.vector.tensor_scalar(
                    msk[:], iota_t[:], lab_t[b][:], BIG, ALU.is_equal, ALU.mult
                )
                if stage == 315:
                    ps_dbg = ps1.tile([1, 1], F32, name="ps_dbg")
                    nc.tensor.matmul(
                        ps_dbg[:], lhsT=msk[:, 0:1], rhs=dists[:, 0:1],
                        start=True, stop=True,
                    )
                    dbg_sb = pp.tile([1, 1], F32, name="dbg_sb")
                    nc.vector.tensor_copy(dbg_sb[:], ps_dbg[:])
                    nc.sync.dma_start(out_t[:, :], dbg_sb[:])
                    return
                ap_t = sp.tile([128, 1], F32, name="ap_t", tag="ap_t")
                an_t = sp.tile([128, 1], F32, name="an_t", tag="an_t")
                if use_ttr:
                    ttro = sp.tile([128, C], F32, name="ttro", tag="ttro")
                    nc.vector.tensor_tensor_reduce(
                        out=ttro[:],
                        in0=dists[:],
                        in1=_f32v(oh_t[b][:], use_f32r),
                        scale=1.0,
                        scalar=MARGIN,
                        op0=ALU.mult,
                        op1=ALU.add,
                        accum_out=ap_t[:],
                    )
                    ttro2 = sp.tile([128, C], F32, name="ttro2", tag="ttro")
                    nc.vector.tensor_tensor_reduce(
                        out=ttro2[:],
                        in0=dists[:],
                        in1=msk[:],
                        scale=1.0,
                        scalar=BIG,
                        op0=ALU.add,
                        op1=ALU.min,
                        accum_out=an_t[:],
                    )
                    # relu(ap - an + margin); margin folded into ap's reduce init
                    nc.vector.tensor_scalar(
                        terms[:, b:b + 1], ap_t[:], an_t[:], 0.0,
                        ALU.subtract, ALU.max,
                    )
                else:
                    ttro = sp.tile([128, C], F32, name="ttro", tag="ttro")
                    nc.vector.tensor_mul(
                        ttro[:], dists[:], _f32v(oh_t[b][:], use_f32r)
                    )
                    nc.vector.reduce_sum(ap_t[:], ttro[:], axis=AX.X)
                    ttro2 = sp.tile([128, C], F32, name="ttro2", tag="ttro")
                    nc.vector.tensor_add(ttro2[:], dists[:], msk[:])
                    nc.vector.tensor_reduce(
                        an_t[:], ttro2[:], axis=AX.X, op=ALU.min
                    )
                    # relu((ap - an) + margin)
                    tsub = sp.tile([128, 1], F32, name="tsub", tag="ap_t")
                    nc.vector.tensor_scalar(
                        tsub[:], ap_t[:], an_t[:], MARGIN, ALU.subtract, ALU.add
                    )
                    nc.vector.tensor_scalar_max(terms[:, b:b + 1], tsub[:], 0.0)
                if stage == 32:
                    _finish(nc, pp, ps1, ap_t[:], ones_col, out_t)
                    return

            acc = pp.tile([128, 1], F32, name="acc")
            nc.vector.reduce_sum(acc[:], terms[:], axis=AX.X)
            ps_loss = ps1.tile([1, 1], F32, name="ps_loss")
            nc.tensor.matmul(
                ps_loss[:], lhsT=acc[:], rhs=ones_col[:], start=True, stop=True
            )
            loss_sb = pp.tile([1, 1], F32, name="loss_sb")
            nc.vector.tensor_copy(loss_sb[:], ps_loss[:])
            nc.sync.dma_start(out_t[:, :], loss_sb[:])


def build(mm="f32", no_collective=False, stage=4, use_ttr=False, ar16=False):
    nc = bacc.Bacc(
        "TRN2",
        target_bir_lowering=False,
        debug=False,
        num_devices=W,
    )
    in_dt = F16 if mm == "f16" else F32
    emb_in = nc.dram_tensor("emb", [NL, D], in_dt, kind="ExternalInput").ap()
    embT_in = nc.dram_tensor("embT", [D, NL], in_dt, kind="ExternalInput").ap()
    lab_in = nc.dram_tensor("labels", [NL, 1], F32, kind="ExternalInput").ap()
    out_t = nc.dram_tensor("loss_partial", [1, 1], F32, kind="ExternalOutput").ap()
    with tile.TileContext(nc) as tc:
        emit(
            nc, tc, emb_in, embT_in, lab_in, out_t,
            mm=mm, no_collective=no_collective, stage=stage,
            use_ttr=use_ttr, ar16=ar16,
        )
    nc.compile()
    return nc


_CACHE = {}


def get_compiled(mm="f32", no_collective=False, stage=4, use_ttr=False, ar16=False):
    key = ("nc", mm, no_collective, stage, use_ttr, ar16)
    if key not in _CACHE:
        _CACHE[key] = build(
            mm=mm, no_collective=no_collective, stage=stage,
            use_ttr=use_ttr, ar16=ar16,
        )
    return _CACHE[key]


def make_in_maps(embeddings, labels, mm="f32"):
    embeddings = np.ascontiguousarray(np.asarray(embeddings), dtype=np.float32)
    labels = np.asarray(labels).astype(np.int32)
    in_dt = np.float16 if mm == "f16" else np.float32
    in_maps = []
    for i in range(W):
        sl = slice(i * NL, (i + 1) * NL)
        e = embeddings[sl]
        in_maps.append(
            {
                "emb": np.ascontiguousarray(e.astype(in_dt)),
                "embT": np.ascontiguousarray(e.T.astype(in_dt)),
                "labels": np.ascontiguousarray(
                    labels[sl].reshape(NL, 1).astype(np.float32)
                ),
            }
        )
    return in_maps


def run(embeddings, labels, mm="f32", trace=False, no_collective=False,
        stage=4, use_ttr=False, ar16=False):
    nc = get_compiled(mm=mm, no_collective=no_collective, stage=stage,
                      use_ttr=use_ttr, ar16=ar16)
    res = run_bass_kernel_spmd(
        nc, make_in_maps(embeddings, labels, mm=mm), core_ids=list(range(W)),
        trace=trace,
    )
    total = sum(float(r["loss_partial"][0, 0]) for r in res.results)
    return np.array(total / N, dtype=np.float32), res


def kernel(embeddings, labels):
    out, _ = run(embeddings, labels)
    return out

